# revision 7
# baseline (speedup 1.0000x reference)
"""Trainium2 Bass kernel for 16-head MHA with RoPE (B=1, S=4096, D=1024).

Sharding: tensor-parallel over heads — 2 heads per core on 8 cores.
Per-core pipeline (all matmuls bf16, fp32 PSUM accumulation):
  1. Load hidden transposed [d, s] (host-prepared bf16) + weight slices.
  2. Projections: k_T/v_T/q_T [c=128, s=4096] with weight chunks stationary.
  3. RoPE on q_T/k_T in fp32 via partition-swap trick (channels permuted
     host-side to [evens | odds] per head so rotation pairs sit 32 apart).
  4. v_T -> DMA-transpose -> v_nat [s, c] -> reshaped into v_ext with a
     ones-column appended per head: per-chunk blocks [v_h0(64)|1|v_h1(64)|1].
  5. Attention per q-tile of 512, per k-chunk of 128 keys:
     - scores transposed S_T[k, q], the two heads row-split on the PE
       array (tile_position (0,0)/(64,0)) into one [128,1024] PSUM pair;
     - one exp on ScalarE per chunk (scale=1/8, mask folded in as the
       per-partition bias vector);
     - ctx matmuls with the 65-wide v_ext lhsT: row 64 of each head's
       ctx PSUM bank accumulates the softmax denominator for free.
  6. Finalize per q-tile (overlapped into the next q-tile via
     double-buffered ctx banks): copy den rows, one [2,512] reciprocal,
     fp32 PE broadcast matmul to all 128 partitions, fused
     normalize+cast to bf16.
  7. Out-projection with ctx_T stationary; fp32 partial written to DRAM.
Host sums the 8 partials.
"""

import functools

import numpy as np
import ml_dtypes

import concourse.bass as bass
import concourse.tile as tile
import concourse.mybir as mybir
from concourse.bass_utils import run_bass_kernel_spmd

BF16 = mybir.dt.bfloat16
F32 = mybir.dt.float32
bf16 = ml_dtypes.bfloat16

S = 4096      # sequence length
D = 1024      # model dim
HD = 64       # head dim
C = 128       # channels per core (2 heads)
NDC = 8       # contraction chunks of 128 over D
NKC = 32      # key chunks of 128 over S
NQT = 8       # query tiles of 512
QT = 512
VW = 2 * (HD + 1)  # v_ext block width per chunk: [v_h0(64)|1|v_h1(64)|1]


_NO_SPLIT = (
    mybir.InstEventSemaphore,
    mybir.InstUnconditionalBranch,
)


def _split_multi_waits(nc: bass.Bass) -> None:
    """Hoist extra sem waits onto standalone EventSemaphore carriers.

    This walrus build only supports one sync-wait command per engine
    instruction ("Too many sync wait commands" in setupSyncWait), so any
    instruction Tile scheduled with >1 wait gets all but its last wait moved
    to dedicated InstEventSemaphore instructions placed immediately before it
    in the same engine stream (sequencer blocks on them in program order —
    semantically identical).
    """
    n = 0
    for fn in nc.m.functions:
        for blk in fn.blocks:
            out = []
            for inst in blk.instructions:
                si = inst.sync_info
                if (
                    si is not None
                    and si.on_wait
                    and len(si.on_wait) > 1
                    and not isinstance(inst, _NO_SPLIT)
                    and inst.engine != mybir.EngineType.Unassigned
                ):
                    waits = list(si.on_wait)
                    for w in waits[:-1]:
                        ev = mybir.InstEventSemaphore(name=f"ant_waitsplit_{n}")
                        n += 1
                        ev.engine = inst.engine
                        ev.sync_info = mybir.SyncInfo(on_wait=[w], on_update=[])
                        nc.register_instruction(ev)
                        out.append(ev)
                    si.on_wait = [waits[-1]]
                    inst.sync_info = si
                out.append(inst)
            blk.instructions[:] = out


def build_program() -> bass.Bass:
    nc = bass.Bass()
    hidT_d = nc.declare_dram_parameter("hidT", [D, S], BF16, isOutput=False)
    wq_d = nc.declare_dram_parameter("wq", [128, D], BF16, isOutput=False)
    wk_d = nc.declare_dram_parameter("wk", [128, D], BF16, isOutput=False)
    wv_d = nc.declare_dram_parameter("wv", [128, D], BF16, isOutput=False)
    wo_d = nc.declare_dram_parameter("wo", [128, D], BF16, isOutput=False)
    cos_d = nc.declare_dram_parameter("cosf", [128, S], F32, isOutput=False)
    sin_d = nc.declare_dram_parameter("sinf", [128, S], F32, isOutput=False)
    mask_d = nc.declare_dram_parameter("maskadd", [128, NKC], F32, isOutput=False)
    sel_d = nc.declare_dram_parameter("sel2", [33, 128], F32, isOutput=False)
    out_d = nc.declare_dram_parameter("outp", [S, D], BF16, isOutput=True)

    Exp = mybir.ActivationFunctionType.Exp
    mult = mybir.AluOpType.mult
    add = mybir.AluOpType.add

    with tile.TileContext(nc) as tc:
        with (
            tc.tile_pool(name="const", bufs=1) as const,
            tc.tile_pool(name="ppool", bufs=3) as ppool,
        ):
            # ---- persistent SBUF tiles -------------------------------------
            wq_sb = const.tile([128, D], BF16, tag="wq")
            wk_sb = const.tile([128, D], BF16, tag="wk")
            wv_sb = const.tile([128, D], BF16, tag="wv")
            wo_sb = const.tile([128, D], BF16, tag="wo")
            mask_sb = const.tile([128, NKC], F32, tag="mask")
            sel_sb = const.tile([33, 128], F32, tag="sel")
            dd_sb = const.tile([33, QT], F32, tag="dd")
            rr_sb = const.tile([33, QT], F32, tag="rr")
            recb_sb = const.tile([128, QT], F32, tag="recb")
            qT_bf = const.tile([128, S], BF16, tag="qTbf")
            kT_bf = const.tile([128, S], BF16, tag="kTbf")
            v_ext = const.tile([128, NKC * VW], BF16, tag="vext")
            ctxn = const.tile([128, S], BF16, tag="ctxn")
            tscratch = const.tile([1, 8], F32, tag="tscratch")

            nc.sync.dma_start(out=wk_sb[:], in_=wk_d[:])
            nc.sync.dma_start(out=wv_sb[:], in_=wv_d[:])
            nc.sync.dma_start(out=wq_sb[:], in_=wq_d[:])
            nc.sync.dma_start(out=mask_sb[:], in_=mask_d[:])
            nc.sync.dma_start(out=sel_sb[:], in_=sel_d[:])
            nc.sync.dma_start(out=wo_sb[:], in_=wo_d[:])
            # ones columns of v_ext (cols HD and 2*HD+1 of each chunk block)
            nc.vector.memset(v_ext[:], 1.0)
            # rows 1..31 of dd stay 1.0 so reciprocal is finite there
            nc.vector.memset(dd_sb[:], 1.0)

            # ---- phase 1: load hidT + projections + rope -------------------
            with (
                tc.tile_pool(name="hid", bufs=1) as hid,
                tc.tile_pool(name="projps", bufs=1, space="PSUM") as projps,
            ):
                hidT_sb = hid.tile([128, NDC * S], BF16, tag="hidT")
                SEG = S // 2
                for dc in range(NDC):
                    nc.sync.dma_start(
                        out=hidT_sb[:, dc * S : (dc + 1) * S],
                        in_=hidT_d[dc * 128 : (dc + 1) * 128, :],
                    )
                qT_f32 = hid.tile([128, S], F32, tag="qTf")
                kT_f32 = hid.tile([128, S], F32, tag="kTf")
                vT_bf = hid.tile([128, S], BF16, tag="vTbf")

                def project(w_sb, dst, dsts=None):
                    ps = [projps.tile([128, QT], F32, name=f"pj{st}", tag=f"pj{st}") for st in range(8)]
                    for dc in range(NDC):
                        for st in range(8):
                            nc.tensor.matmul(
                                ps[st][:],
                                lhsT=w_sb[:, dc * 128 : (dc + 1) * 128],
                                rhs=hidT_sb[:, dc * S + st * QT : dc * S + (st + 1) * QT],
                                start=(dc == 0),
                                stop=(dc == NDC - 1),
                            )
                    # evacuate on ScalarE (idle during phase 1); DVE does rope
                    for st in range(8):
                        nc.scalar.copy(dst[:, st * QT : (st + 1) * QT], ps[st][:])

                project(wv_sb, vT_bf)
                # v_T [c, s] -> v_nat [s, c] (32 transposed chunks), then
                # spread into v_ext blocks [v_h0(64)|1|v_h1(64)|1] on ScalarE
                # (the DMA-transpose writes blocks contiguously; the ones
                # columns come from the earlier memset).
                v_nat = hid.tile([128, S], BF16, tag="vnat")
                nc.sync.dma_start_transpose(
                    out=v_nat[:].rearrange("p (kc c) -> p kc c", kc=NKC),
                    in_=vT_bf[:],
                )
                for h in range(2):
                    nc.scalar.copy(
                        v_ext[:].rearrange("p (kc w) -> p kc w", kc=NKC)[
                            :, :, h * (HD + 1) : h * (HD + 1) + HD
                        ],
                        v_nat[:].rearrange("p (kc c) -> p kc c", kc=NKC)[
                            :, :, h * HD : (h + 1) * HD
                        ],
                    )
                project(wk_sb, kT_f32)
                project(wq_sb, qT_f32)

                # rope streamed in s-segments to bound SBUF: channel rows per
                # head h: [h*64, h*64+32) = even channels ("a"),
                # [h*64+32, h*64+64) = odd ("b");
                # out = x * cos_full + swap(x) * sin_signed
                with tc.tile_pool(name="ropep", bufs=2) as ropep:
                    for seg in range(2):
                        sc = slice(seg * SEG, (seg + 1) * SEG)
                        cos_sb = ropep.tile([128, SEG], F32, tag="cs")
                        sin_sb = ropep.tile([128, SEG], F32, tag="sn")
                        nc.sync.dma_start(out=cos_sb[:], in_=cos_d[:, sc])
                        nc.sync.dma_start(out=sin_sb[:], in_=sin_d[:, sc])
                        # touch ops absorb the DMA waits on DVE so the rope
                        # tensor_tensor ops stay within the 1-wait TT limit
                        nc.vector.tensor_copy(tscratch[0:1, 0:1], cos_sb[0:1, 0:1])
                        nc.vector.tensor_copy(tscratch[0:1, 1:2], sin_sb[0:1, 0:1])
                        for x_f32, out_bf in ((kT_f32, kT_bf), (qT_f32, qT_bf)):
                            qsw = ropep.tile([128, SEG], F32, tag="qsw", bufs=1)
                            for h in range(2):
                                a = slice(h * 64, h * 64 + 32)
                                b = slice(h * 64 + 32, h * 64 + 64)
                                nc.vector.tensor_copy(qsw[a, :], x_f32[b, sc])
                                nc.vector.tensor_copy(qsw[b, :], x_f32[a, sc])
                            nc.vector.tensor_tensor(
                                x_f32[:, sc], x_f32[:, sc], cos_sb[:], op=mult
                            )
                            nc.vector.tensor_tensor(qsw[:], qsw[:], sin_sb[:], op=mult)
                            nc.vector.tensor_tensor(
                                out_bf[:, sc], x_f32[:, sc], qsw[:], op=add
                            )

            # ---- phase 2: attention ---------------------------------------
            with (
                tc.tile_pool(name="sgps", bufs=2, space="PSUM") as sgps,
                tc.tile_pool(name="ctxps", bufs=2, space="PSUM") as ctxps,
            ):
                # finalize state carried across qtile boundaries
                pending = None  # (ctxA, ctxB, qc)

                def finalize_head(ctxA, ctxB, qt):
                    # den rows (at partitions 0 and 32) -> reciprocal (on DVE)
                    nc.vector.tensor_copy(dd_sb[0:1, :], ctxA[64:65, :])
                    nc.vector.tensor_copy(dd_sb[32:33, :], ctxB[64:65, :])
                    nc.vector.reciprocal(rr_sb[:], dd_sb[:])

                def finalize_tail(ctxA, ctxB, qc):
                    # broadcast 1/den to all partitions via fp32 PE matmul
                    recb = sgps.tile([128, QT], F32, tag="sg")
                    nc.tensor.matmul(
                        recb[:], lhsT=sel_sb[:], rhs=rr_sb[:], start=True, stop=True
                    )
                    nc.vector.tensor_copy(recb_sb[:], recb[:])
                    nc.vector.tensor_tensor(
                        ctxn[0:64, qc], ctxA[0:64, :], recb_sb[0:64, :], op=mult
                    )
                    nc.vector.tensor_tensor(
                        ctxn[64:128, qc], ctxB[0:64, :], recb_sb[64:128, :], op=mult
                    )

                for qt in range(NQT):
                    qc = slice(qt * QT, (qt + 1) * QT)
                    ctxA = ctxps.tile([128, QT], F32, tag="ctxA")
                    ctxB = ctxps.tile([128, QT], F32, tag="ctxB")
                    for c in range(NKC):
                        if c == 10 and pending is not None:
                            finalize_tail(*pending)
                            pending = None
                        sg = sgps.tile([128, 2 * QT], F32, tag="sg")
                        Pt = ppool.tile([128, 2 * QT], BF16, tag="pt")
                        for h in range(2):
                            hr = slice(h * 64, (h + 1) * 64)
                            nc.tensor.matmul(
                                sg[:, h * QT : (h + 1) * QT],
                                lhsT=kT_bf[hr, c * 128 : (c + 1) * 128],
                                rhs=qT_bf[hr, qc],
                                start=True,
                                stop=True,
                            )
                        nc.scalar.activation(
                            Pt[:], sg[:], Exp, bias=mask_sb[:, c : c + 1], scale=0.125
                        )
                        for h, ctx_ps in ((0, ctxA), (1, ctxB)):
                            nc.tensor.matmul(
                                ctx_ps[0:65, :],
                                lhsT=v_ext[
                                    :, c * VW + h * (HD + 1) : c * VW + (h + 1) * (HD + 1)
                                ],
                                rhs=Pt[:, h * QT : (h + 1) * QT],
                                start=(c == 0),
                                stop=(c == NKC - 1),
                            )
                    finalize_head(ctxA, ctxB, qt)
                    pending = (ctxA, ctxB, qc)
                finalize_tail(*pending)

            # ---- phase 3: output projection -------------------------------
            with (
                tc.tile_pool(name="ops", bufs=3, space="PSUM") as ops_pool,
                tc.tile_pool(name="outsb", bufs=3) as outsb_pool,
            ):
                for i in range(32):
                    ops_ = ops_pool.tile([128, D], F32, tag="ops")
                    for j in range(2):
                        nc.tensor.matmul(
                            ops_[:, j * QT : (j + 1) * QT],
                            lhsT=ctxn[:, i * 128 : (i + 1) * 128],
                            rhs=wo_sb[:, j * QT : (j + 1) * QT],
                            start=True,
                            stop=True,
                        )
                    osb = outsb_pool.tile([128, D], BF16, tag="osb")
                    if i % 2 == 0:
                        nc.scalar.copy(osb[:], ops_[:])
                    else:
                        nc.vector.tensor_copy(osb[:], ops_[:])
                    nc.sync.dma_start(
                        out=out_d[i * 128 : (i + 1) * 128, :], in_=osb[:]
                    )

    _split_multi_waits(nc)
    return nc


@functools.cache
def _cached_program() -> bass.Bass:
    return build_program()


def _prep_inputs(hidden_states, freqs_cis, attention_mask, wq, wk, wv, wo):
    hid = np.asarray(hidden_states, np.float32).reshape(S, D)
    hidT = np.ascontiguousarray(hid.T).astype(bf16)

    # within-head channel permutation: evens then odds (rope pairs 32 apart)
    perm1 = np.concatenate([np.arange(0, HD, 2), np.arange(1, HD, 2)])
    perm = np.concatenate([perm1, perm1 + HD])  # for the 2 heads of a core

    fc = np.asarray(freqs_cis, np.float32)
    cosT = np.ascontiguousarray(fc[:, :, 0].T)  # [32, S]
    sinT = np.ascontiguousarray(fc[:, :, 1].T)
    cosf = np.concatenate([cosT, cosT, cosT, cosT], 0).astype(np.float32)
    sinf = np.concatenate([-sinT, sinT, -sinT, sinT], 0).astype(np.float32)

    mask_add = (1.0 - np.asarray(attention_mask, np.float32).reshape(S)) * -10000.0
    maskadd = np.ascontiguousarray(mask_add.reshape(NKC, 128).T).astype(np.float32)

    sel2 = np.zeros((33, 128), np.float32)
    sel2[0, 0:64] = 1.0
    sel2[32, 64:128] = 1.0

    def wlayout(w):  # [1024, 128] -> [128 partitions, chunk-major 1024]
        w = np.ascontiguousarray(w)
        return np.ascontiguousarray(
            w.reshape(NDC, 128, 128).transpose(1, 0, 2).reshape(128, D)
        ).astype(bf16)

    in_maps = []
    for core in range(8):
        cols = slice(core * 128, (core + 1) * 128)
        in_maps.append(
            {
                "hidT": hidT,
                "wq": wlayout(np.asarray(wq, np.float32)[:, cols][:, perm]),
                "wk": wlayout(np.asarray(wk, np.float32)[:, cols][:, perm]),
                "wv": wlayout(np.asarray(wv, np.float32)[:, cols]),
                "wo": np.ascontiguousarray(np.asarray(wo, np.float32)[cols, :]).astype(bf16),
                "cosf": cosf,
                "sinf": sinf,
                "maskadd": maskadd,
                "sel2": sel2,
            }
        )
    return in_maps


def run_sharded(in_maps, **kwargs):
    nc = _cached_program()
    return run_bass_kernel_spmd(nc, in_maps, list(range(8)), **kwargs)


def kernel(hidden_states, freqs_cis, attention_mask, wq, wk, wv, wo):
    in_maps = _prep_inputs(
        hidden_states, freqs_cis, attention_mask, wq, wk, wv, wo
    )
    res = run_sharded(in_maps).results
    out = np.zeros((S, D), np.float32)
    for r in res:
        out += np.asarray(r["outp"], np.float32)
    return out.reshape(1, S, D)


if __name__ == "__main__":
    import reference

    inputs = reference.setup_inputs()
    inputs = {k: np.asarray(v) for k, v in inputs.items()}
    expected = np.asarray(reference.reference(**inputs))
    actual = kernel(**inputs)
    err = np.abs(actual - expected).max() / np.abs(expected).max()
    print("Relative error:", err)


# revision 15
# speedup vs baseline: 1.0177x; 1.0177x over previous
"""Trainium2 Bass kernel for 16-head MHA with RoPE (B=1, S=4096, D=1024).

Sharding: tensor-parallel over heads — 2 heads per core on 8 cores.
Fully-fused per-core pipeline (all matmuls bf16, fp32 PSUM accumulation):

  Producer chains, one per s-tile of 512 (emitted ahead of the consumer):
    DMA hidT s-block + cos/sin seg -> projections k/v/q (weight chunks
    stationary, 2-bank PSUM ring) -> DVE evac -> RoPE on q/k (partition-
    swap trick; channels permuted host-side so rotation pairs sit 32
    apart) -> v DMA-transposed and spread into v_ext chunk blocks
    [v_h0(64)|1|v_h1(64)|1] (ones columns from a prior memset).

  Attention, per q-tile of 512, per k-chunk of 128 keys:
    - transposed scores S_T[k, q]: the two heads row-split on the PE
      array into one [128,1024] PSUM pair;
    - one exp per chunk on ScalarE (scale=1/8, mask folded into the
      per-partition bias vector);
    - ctx matmuls with the 65-wide v_ext lhsT: row 64 of each head's
      ctx bank accumulates the softmax denominator for free.
    Finalize (overlapped into the next q-tile): den rows -> reciprocal
    (fp16) -> fp16 PE broadcast matmul -> fused normalize+cast to bf16.
    Out-projection (wo stationary, output transposed [D, S]) is
    distributed into the next q-tile's chunk loop; host transposes and
    sums the 8 bf16 partials.
"""

import functools

import numpy as np
import ml_dtypes

import concourse.bass as bass
import concourse.tile as tile
import concourse.mybir as mybir
from concourse.bass_utils import run_bass_kernel_spmd

BF16 = mybir.dt.bfloat16
F16 = mybir.dt.float16
F32 = mybir.dt.float32
bf16 = ml_dtypes.bfloat16

S = 4096      # sequence length
D = 1024      # model dim
HD = 64       # head dim
NDC = 8       # contraction chunks of 128 over D
NKC = 32      # key chunks of 128 over S
NQT = 8       # query tiles of 512
QT = 512
VW = 2 * (HD + 1)  # v_ext block width per chunk: [v_h0(64)|1|v_h1(64)|1]


_NO_SPLIT = (
    mybir.InstEventSemaphore,
    mybir.InstUnconditionalBranch,
    mybir.InstISA,
)


def _split_multi_waits(nc: bass.Bass) -> None:
    """Hoist extra sem waits onto standalone EventSemaphore carriers.

    This walrus build only supports one sync-wait command per engine
    instruction ("Too many sync wait commands" in setupSyncWait), so any
    instruction Tile scheduled with >1 wait gets all but its last wait moved
    to dedicated InstEventSemaphore instructions placed immediately before it
    in the same engine stream (sequencer blocks on them in program order —
    semantically identical).
    """
    n = 0
    for fn in nc.m.functions:
        for blk in fn.blocks:
            out = []
            for inst in blk.instructions:
                si = inst.sync_info
                if (
                    si is not None
                    and si.on_wait
                    and len(si.on_wait) > 1
                    and not isinstance(inst, _NO_SPLIT)
                    and inst.engine != mybir.EngineType.Unassigned
                ):
                    waits = list(si.on_wait)
                    for w in waits[:-1]:
                        ev = mybir.InstEventSemaphore(name=f"ant_waitsplit_{n}")
                        n += 1
                        ev.engine = inst.engine
                        ev.sync_info = mybir.SyncInfo(on_wait=[w], on_update=[])
                        nc.register_instruction(ev)
                        out.append(ev)
                    si.on_wait = [waits[-1]]
                    inst.sync_info = si
                out.append(inst)
            blk.instructions[:] = out


def build_program() -> bass.Bass:
    nc = bass.Bass()
    hidT_d = nc.declare_dram_parameter("hidT", [D, S], BF16, isOutput=False)
    wq_d = nc.declare_dram_parameter("wq", [128, D], BF16, isOutput=False)
    wk_d = nc.declare_dram_parameter("wk", [128, D], BF16, isOutput=False)
    wv_d = nc.declare_dram_parameter("wv", [128, D], BF16, isOutput=False)
    wo_d = nc.declare_dram_parameter("wo", [128, D], BF16, isOutput=False)
    cos_d = nc.declare_dram_parameter("cosf", [128, S], F32, isOutput=False)
    sin_d = nc.declare_dram_parameter("sinf", [128, S], F32, isOutput=False)
    mask_d = nc.declare_dram_parameter("maskadd", [128, NKC], F32, isOutput=False)
    sel_d = nc.declare_dram_parameter("sel2", [33, 128], F16, isOutput=False)
    out_d = nc.declare_dram_parameter("outp", [D, S], BF16, isOutput=True)

    Exp = mybir.ActivationFunctionType.Exp
    mult = mybir.AluOpType.mult
    add = mybir.AluOpType.add

    with tile.TileContext(nc) as tc:
        with (
            tc.tile_pool(name="const", bufs=1) as const,
            tc.tile_pool(name="ppool", bufs=8) as ppool,
            tc.tile_pool(name="hid", bufs=1) as hid,
            tc.tile_pool(name="ropep", bufs=2) as ropep,
            tc.tile_pool(name="outsb", bufs=2) as outsb_pool,
            tc.tile_pool(name="pjps", bufs=2, space="PSUM") as pjps,
            tc.tile_pool(name="sgps", bufs=2, space="PSUM") as sgps,
            tc.tile_pool(name="ctxps", bufs=1, space="PSUM") as ctxps,
        ):
            # ---- persistent SBUF tiles -------------------------------------
            wq_sb = const.tile([128, D], BF16, tag="wq")
            wk_sb = const.tile([128, D], BF16, tag="wk")
            wv_sb = const.tile([128, D], BF16, tag="wv")
            wo_sb = const.tile([128, D], BF16, tag="wo")
            mask_sb = const.tile([128, NKC], F32, tag="mask")
            sel_sb = const.tile([33, 128], F16, tag="sel")
            dd_sb = const.tile([33, QT], F32, tag="dd")
            rr_sb = const.tile([33, QT], F16, tag="rr")
            recb_sb = const.tile([128, QT], F32, tag="recb")
            qT_bf = const.tile([128, S], BF16, tag="qTbf")
            kT_bf = const.tile([128, S], BF16, tag="kTbf")
            v_ext = const.tile([128, NKC * VW], BF16, tag="vext")
            ctxn = const.tile([128, S], BF16, tag="ctxn")
            tscratch = const.tile([1, 8], F32, tag="tscratch")
            hidT_sb = hid.tile([128, NDC * S], BF16, tag="hidT")

            nc.sync.dma_start(out=wk_sb[:], in_=wk_d[:])
            nc.sync.dma_start(out=wv_sb[:], in_=wv_d[:])
            nc.sync.dma_start(out=wq_sb[:], in_=wq_d[:])
            nc.sync.dma_start(out=mask_sb[:], in_=mask_d[:])
            nc.sync.dma_start(out=sel_sb[:], in_=sel_d[:])
            nc.sync.dma_start(out=wo_sb[:], in_=wo_d[:])
            # ones columns of v_ext (cols HD and 2*HD+1 of each chunk block)
            nc.vector.memset(v_ext[:], 1.0)
            # rows 1..31 of dd stay 1.0 so reciprocal is finite there
            nc.vector.memset(dd_sb[:], 1.0)

            vext3 = v_ext[:].rearrange("p (kc w) -> p kc w", kc=NKC)

            # ---- producer chain for one s-tile of 512 ----------------------
            def emit_st(st):
                sc = slice(st * QT, (st + 1) * QT)
                for dc in range(NDC):
                    nc.sync.dma_start(
                        out=hidT_sb[:, dc * S + st * QT : dc * S + (st + 1) * QT],
                        in_=hidT_d[dc * 128 : (dc + 1) * 128, sc],
                    )
                cos_sb = ropep.tile([128, QT], F32, tag="cs")
                sin_sb = ropep.tile([128, QT], F32, tag="sn")
                nc.sync.dma_start(out=cos_sb[:], in_=cos_d[:, sc])
                nc.sync.dma_start(out=sin_sb[:], in_=sin_d[:, sc])
                nc.vector.tensor_copy(tscratch[0:1, 0:1], cos_sb[0:1, 0:1])
                nc.vector.tensor_copy(tscratch[0:1, 1:2], sin_sb[0:1, 0:1])

                vT_t = ropep.tile([128, QT], BF16, tag="vT")
                kf_t = ropep.tile([128, QT], F32, tag="kf")
                qf_t = ropep.tile([128, QT], F32, tag="qf")
                for w_sb, dst in ((wk_sb, kf_t), (wv_sb, vT_t), (wq_sb, qf_t)):
                    ps = pjps.tile([128, QT], F32, tag="pj")
                    for dc in range(NDC):
                        nc.tensor.matmul(
                            ps[:],
                            lhsT=w_sb[:, dc * 128 : (dc + 1) * 128],
                            rhs=hidT_sb[
                                :, dc * S + st * QT : dc * S + (st + 1) * QT
                            ],
                            start=(dc == 0),
                            stop=(dc == NDC - 1),
                        )
                    nc.vector.tensor_copy(dst[:], ps[:])

                # v: transpose the 4 key-chunks of this s-tile, then spread
                # into v_ext blocks (ones columns preserved from the memset)
                v_nat = ropep.tile([128, QT], BF16, tag="vnat")
                nc.sync.dma_start_transpose(
                    out=v_nat[:].rearrange("p (kc c) -> p kc c", kc=4),
                    in_=vT_t[:],
                )
                vnat3 = v_nat[:].rearrange("p (kc c) -> p kc c", kc=4)
                for h in range(2):
                    nc.vector.tensor_copy(
                        vext3[
                            :, 4 * st : 4 * st + 4, h * (HD + 1) : h * (HD + 1) + HD
                        ],
                        vnat3[:, :, h * HD : (h + 1) * HD],
                    )

                # rope: channel rows per head h: [h*64, h*64+32) = even
                # channels ("a"), [h*64+32, h*64+64) = odd ("b");
                # out = x * cos_full + swap(x) * sin_signed
                for x_f32, out_bf in ((kf_t, kT_bf), (qf_t, qT_bf)):
                    qsw = ropep.tile([128, QT], F32, tag="qsw")
                    for h in range(2):
                        a = slice(h * 64, h * 64 + 32)
                        b = slice(h * 64 + 32, h * 64 + 64)
                        nc.vector.tensor_copy(qsw[a, :], x_f32[b, :])
                        nc.vector.tensor_copy(qsw[b, :], x_f32[a, :])
                    nc.vector.tensor_tensor(x_f32[:], x_f32[:], cos_sb[:], op=mult)
                    nc.vector.tensor_tensor(qsw[:], qsw[:], sin_sb[:], op=mult)
                    nc.vector.tensor_tensor(out_bf[:, sc], x_f32[:], qsw[:], op=add)

            # ---- attention helpers -----------------------------------------
            def finalize_head(ctxA, ctxB):
                # den rows (partitions 0/32 of dd) -> fast 1/x (~18 bits),
                # cast to fp16 for the single-pass broadcast matmul
                nc.vector.tensor_copy(dd_sb[0:1, :], ctxA[64:65, :])
                nc.vector.tensor_copy(dd_sb[32:33, :], ctxB[64:65, :])
                with nc.allow_low_precision(reason="1/den broadcast in fp16"):
                    nc.vector.reciprocal(rr_sb[:], dd_sb[:])

            def finalize_tail(ctxA, ctxB, qc):
                # broadcast 1/den to all partitions via fp16 PE matmul
                recb = sgps.tile([128, QT], F32, tag="sg")
                nc.tensor.matmul(
                    recb[:], lhsT=sel_sb[:], rhs=rr_sb[:], start=True, stop=True
                )
                nc.vector.tensor_copy(recb_sb[:], recb[:])
                nc.vector.tensor_tensor(
                    ctxn[0:64, qc], ctxA[0:64, :], recb_sb[0:64, :], op=mult
                )
                nc.vector.tensor_tensor(
                    ctxn[64:128, qc], ctxB[0:64, :], recb_sb[64:128, :], op=mult
                )

            def emit_outproj(t, obs):
                # out-projection of q-tile t, output transposed: wo stationary
                tc_ = slice(t * QT, (t + 1) * QT)
                for ob in obs:
                    ops_ = pjps.tile([128, QT], F32, tag="pj")
                    nc.tensor.matmul(
                        ops_[:],
                        lhsT=wo_sb[:, ob * 128 : (ob + 1) * 128],
                        rhs=ctxn[:, tc_],
                        start=True,
                        stop=True,
                    )
                    osb = outsb_pool.tile([128, QT], BF16, tag="osb")
                    nc.vector.tensor_copy(osb[:], ops_[:])
                    nc.sync.dma_start(
                        out=out_d[ob * 128 : (ob + 1) * 128, tc_], in_=osb[:]
                    )

            # ---- fused emission --------------------------------------------
            def emit_ctx(c, Pt, ctxA, ctxB):
                for h, ctx_ps in ((0, ctxA), (1, ctxB)):
                    nc.tensor.matmul(
                        ctx_ps[0:65, :],
                        lhsT=v_ext[
                            :,
                            c * VW + h * (HD + 1) : c * VW + (h + 1) * (HD + 1),
                        ],
                        rhs=Pt[:, h * QT : (h + 1) * QT],
                        start=(c == 0),
                        stop=(c == NKC - 1),
                    )

            W = 6  # leading chunks whose ctx waits for the prior finalize
            emit_st(0)
            emit_st(1)
            pending = None
            for qt in range(NQT):
                qc = slice(qt * QT, (qt + 1) * QT)
                ctxA = ctxps.tile([128, QT], F32, tag="ctxA")
                ctxB = ctxps.tile([128, QT], F32, tag="ctxB")
                deferred = []
                for c in range(NKC):
                    if qt == 0 and c >= 4 and c % 4 == 0 and c // 4 + 1 <= 7:
                        emit_st(c // 4 + 1)
                    if qt > 1 and c in (12, 14, 16, 18):
                        ob0 = c - 12
                        emit_outproj(qt - 2, (ob0, ob0 + 1))
                    sg = sgps.tile([128, 2 * QT], F32, tag="sg")
                    Pt = ppool.tile([128, 2 * QT], BF16, tag="pt")
                    for h in range(2):
                        hr = slice(h * 64, (h + 1) * 64)
                        nc.tensor.matmul(
                            sg[:, h * QT : (h + 1) * QT],
                            lhsT=kT_bf[hr, c * 128 : (c + 1) * 128],
                            rhs=qT_bf[hr, qc],
                            start=True,
                            stop=True,
                        )
                    nc.scalar.activation(
                        Pt[:], sg[:], Exp, bias=mask_sb[:, c : c + 1], scale=0.125
                    )
                    if pending is not None and c < W:
                        deferred.append((c, Pt))
                        if c == W - 1:
                            finalize_tail(*pending)
                            pending = None
                            for cc, Ptc in deferred:
                                emit_ctx(cc, Ptc, ctxA, ctxB)
                            deferred = []
                    else:
                        emit_ctx(c, Pt, ctxA, ctxB)
                finalize_head(ctxA, ctxB)
                pending = (ctxA, ctxB, qc)
            finalize_tail(*pending)
            for t in (NQT - 2, NQT - 1):
                emit_outproj(t, range(8))

    _split_multi_waits(nc)
    return nc


@functools.cache
def _cached_program() -> bass.Bass:
    return build_program()


def _prep_inputs(hidden_states, freqs_cis, attention_mask, wq, wk, wv, wo):
    hid = np.asarray(hidden_states, np.float32).reshape(S, D)
    hidT = np.ascontiguousarray(hid.T).astype(bf16)

    # within-head channel permutation: evens then odds (rope pairs 32 apart)
    perm1 = np.concatenate([np.arange(0, HD, 2), np.arange(1, HD, 2)])
    perm = np.concatenate([perm1, perm1 + HD])  # for the 2 heads of a core

    fc = np.asarray(freqs_cis, np.float32)
    cosT = np.ascontiguousarray(fc[:, :, 0].T)  # [32, S]
    sinT = np.ascontiguousarray(fc[:, :, 1].T)
    cosf = np.concatenate([cosT, cosT, cosT, cosT], 0).astype(np.float32)
    sinf = np.concatenate([-sinT, sinT, -sinT, sinT], 0).astype(np.float32)

    mask_add = (1.0 - np.asarray(attention_mask, np.float32).reshape(S)) * -10000.0
    maskadd = np.ascontiguousarray(mask_add.reshape(NKC, 128).T).astype(np.float32)

    sel2 = np.zeros((33, 128), np.float16)
    sel2[0, 0:64] = 1.0
    sel2[32, 64:128] = 1.0

    def wlayout(w):  # [1024, 128] -> [128 partitions, chunk-major 1024]
        w = np.ascontiguousarray(w)
        return np.ascontiguousarray(
            w.reshape(NDC, 128, 128).transpose(1, 0, 2).reshape(128, D)
        ).astype(bf16)

    in_maps = []
    for core in range(8):
        cols = slice(core * 128, (core + 1) * 128)
        in_maps.append(
            {
                "hidT": hidT,
                "wq": wlayout(np.asarray(wq, np.float32)[:, cols][:, perm]),
                "wk": wlayout(np.asarray(wk, np.float32)[:, cols][:, perm]),
                "wv": wlayout(np.asarray(wv, np.float32)[:, cols]),
                "wo": np.ascontiguousarray(np.asarray(wo, np.float32)[cols, :]).astype(bf16),
                "cosf": cosf,
                "sinf": sinf,
                "maskadd": maskadd,
                "sel2": sel2,
            }
        )
    return in_maps


def run_sharded(in_maps, **kwargs):
    nc = _cached_program()
    return run_bass_kernel_spmd(nc, in_maps, list(range(8)), **kwargs)


def kernel(hidden_states, freqs_cis, attention_mask, wq, wk, wv, wo):
    in_maps = _prep_inputs(
        hidden_states, freqs_cis, attention_mask, wq, wk, wv, wo
    )
    res = run_sharded(in_maps).results
    out = np.zeros((D, S), np.float32)
    for r in res:
        out += np.asarray(r["outp"], np.float32)
    return np.ascontiguousarray(out.T).reshape(1, S, D)


if __name__ == "__main__":
    import reference

    inputs = reference.setup_inputs()
    inputs = {k: np.asarray(v) for k, v in inputs.items()}
    expected = np.asarray(reference.reference(**inputs))
    actual = kernel(**inputs)
    err = np.abs(actual - expected).max() / np.abs(expected).max()
    print("Relative error:", err)
